# revision 9
# baseline (speedup 1.0000x reference)
"""Trainium2 Bass kernel for nn_CoOccurrenceGraph.

Computation (full problem: B=64, C=512, D=1024):
    ew  = edge_weights(co_occurrence, class_counts, context_embeddings)  # [C,C]
    x_t = ew @ x[b]                          # per batch
    gate = sigmoid(sum(x*x_t, -1)/sqrt(D))   # [B,C,1]
    out  = x*(1-gate) + x_t*gate

Strategy: data-parallel over batch across 8 NeuronCores (8 batches/core).
Each core builds the full [C,C] edge-weight matrix on-device (replicated),
then runs the per-batch matmuls + gating.

V2 design (vs the 193 us baseline):
  * Host precomputes the tiny count-derived factor
    balu = 2.5 * balance * u_i * u_j * offdiag  ([C,C], ~1MB, setup-scale)
    and the normalized embeddings, killing most of the edge-weight build's
    on-device elementwise work and ACT-table thrash.
  * Per-chunk (128-row) edge-weight pipeline: the first chunk's weights
    reach the PE ~10us in; stage-B matmuls then overlap the remaining
    edge-weight build.
  * Softmax row-scaling rides the PE "transpose" as a matmul against
    diag(0.9/rowsum) - no separate scale pass.
  * sigmoid(z) = (tanh(... )+1)/2 keeps the whole affinity+confidence
    group in the sigmoid/tanh ACT table set; exp/softmax is the only
    other set -> 5 table loads total (vs 8), none on the critical path.
  * d = (ew - I)@x is copied PSUM->SBUF as bf16 so both DVE passes
    (q-accum and the gated combine) run in 2x packed mode.
  * Output is written bf16 (host casts back to f32): halves output DMA.
  * ss = sum(x^2) runs on ACT (Square, set-filler) for m==0 tiles and on
    DVE for the rest to balance the two engines.
"""

import numpy as np

import concourse.bass as bass
import concourse.bacc as bacc
import concourse.mybir as mybir
import concourse.tile as tile
from concourse.bass_utils import run_bass_kernel_spmd

F32 = mybir.dt.float32
BF16 = mybir.dt.bfloat16
AX = mybir.AxisListType
OP = mybir.AluOpType
AF = mybir.ActivationFunctionType

B, C, D = 64, 512, 1024
P = 128
NCORES = 8
BPC = B // NCORES          # batches per core
CT = C // P                # 4 chunks of 128 rows
NT = D // 512              # 2 matmul n-groups
SMOOTH = 0.01
INV32 = 1.0 / float(np.sqrt(D))      # 1/32
SQ_SCALE = float(D) ** -0.25         # Square(s*x) accumulates x^2/sqrt(D)

_CACHE = {}


def _build_module():
    nc = bacc.Bacc("TRN2", target_bir_lowering=False, debug=False,
                   num_devices=NCORES)
    x_d = nc.dram_tensor("x", [BPC, C, D], BF16, kind="ExternalInput").ap()
    co_d = nc.dram_tensor("co", [C, C], F32, kind="ExternalInput").ap()
    balu_d = nc.dram_tensor("balu", [C, C], F32, kind="ExternalInput").ap()
    nemb_d = nc.dram_tensor("nemb", [C, 4], F32, kind="ExternalInput").ap()
    id_d = nc.dram_tensor("ident", [P, P], F32, kind="ExternalInput").ap()
    y_d = nc.dram_tensor("y", [BPC, C, D], BF16, kind="ExternalOutput").ap()

    with tile.TileContext(nc) as tc:
        _body(nc, tc, x_d, co_d, balu_d, nemb_d, id_d, y_d)
    if not nc.is_finalized():
        nc.finalize()
    return nc


def _body(nc, tc, x_d, co_d, balu_d, nemb_d, id_d, y_d):
    from contextlib import ExitStack
    with ExitStack() as ctx:
        persist = ctx.enter_context(tc.tile_pool(name="persist", bufs=1))
        work = ctx.enter_context(tc.tile_pool(name="work", bufs=2))
        tiny = ctx.enter_context(tc.tile_pool(name="tiny", bufs=6))
        xbp = ctx.enter_context(tc.tile_pool(name="xb", bufs=32))
        dsp = ctx.enter_context(tc.tile_pool(name="ds", bufs=6))
        gbp = ctx.enter_context(tc.tile_pool(name="gb", bufs=3))
        obp = ctx.enter_context(tc.tile_pool(name="ob", bufs=4))
        tbp = ctx.enter_context(tc.tile_pool(name="tb", bufs=10))
        psS = ctx.enter_context(
            tc.tile_pool(name="psS", bufs=2, space="PSUM"))   # sim [P,C]
        psT = ctx.enter_context(
            tc.tile_pool(name="psT", bufs=2, space="PSUM"))   # tr [P,P]
        psB = ctx.enter_context(
            tc.tile_pool(name="psB", bufs=2, space="PSUM"))   # d [P,D]

        # ---------------- input DMAs (order = priority) ----------------
        id_t = persist.tile([P, P], F32, tag="ident")
        nc.sync.dma_start(id_t[:], id_d[:])
        e_t = []
        for c in range(CT):
            et = tiny.tile([P, 4], F32, tag=f"emb{c}")
            nc.sync.dma_start(et[:], nemb_d[bass.ts(c, P), :])
            e_t.append(et)
        co_t = []
        balu_t = []
        for c in range(CT):
            ct_ = persist.tile([P, C], F32, tag=f"co{c}")
            nc.sync.dma_start(ct_[:], co_d[bass.ts(c, P), :])
            co_t.append(ct_)
        for c in range(CT):
            bt_ = persist.tile([P, C], F32, tag=f"balu{c}")
            nc.sync.dma_start(bt_[:], balu_d[bass.ts(c, P), :])
            balu_t.append(bt_)
        xt_all = []
        for b in range(BPC):
            xt = []
            for k in range(CT):
                xk = xbp.tile([P, D], BF16, tag="x")
                nc.sync.dma_start(xk[:], x_d[b, bass.ts(k, P), :])
                xt.append(xk)
            xt_all.append(xt)

        # eye09 = 0.9*I (for the diagonal-block fix of A = 0.9*(sm - I))
        eye09 = persist.tile([P, P], F32, tag="eye09")
        nc.vector.tensor_scalar(eye09[:], id_t[:], 0.9, None, OP.mult)
        bm25 = persist.tile([P, 1], F32, tag="bm25")
        nc.vector.memset(bm25[:], -2.5)

        # nembT [4, C]: PE transposes of the host-normalized embeddings
        nembT = persist.tile([4, C], F32, tag="nembT")
        for c in range(CT):
            neT_ps = psT.tile([4, P], F32, tag="tr")
            nc.tensor.transpose(neT_ps[:], e_t[c][:], id_t[:])
            nc.scalar.copy(nembT[:, bass.ts(c, P)], neT_ps[:])

        # per-chunk stage-E state
        sim_sb = [None] * CT
        th_t = [None] * CT
        conf_t = [None] * CT
        E_t = [None] * CT
        rs_t = [None] * CT
        BtAll = persist.tile([P, CT * C], BF16, tag="BtAll")

        def sim_chunk(c):
            # sim_c stays in PSUM; read by ACT (th) and DVE (aff)
            s_ps = psS.tile([P, C], F32, tag="sim")
            nc.tensor.matmul(s_ps[:], nembT[:, bass.ts(c, P)], nembT[:],
                             start=True, stop=True)
            sim_sb[c] = s_ps

        def th_conf_chunk(c):
            # th = tanh(5*sim - 2.5); sigmoid(10(sim-.5)) = (th+1)/2
            th = work.tile([P, C], F32, tag="th", bufs=4)
            nc.scalar.activation(th[:], sim_sb[c][:], AF.Tanh,
                                 bias=bm25[:], scale=5.0)
            th_t[c] = th
            cf = work.tile([P, C], F32, tag="conf", bufs=4)
            nc.scalar.activation(cf[:], co_t[c][:], AF.Tanh, scale=0.1)
            conf_t[c] = cf

        def pre_chunk(c):
            # aff2 = (th+1)*sim ; ncoB = (co+s)*balu ; pre = ncoB*aff2*conf
            aff2 = work.tile([P, C], F32, tag="aff2")
            nc.vector.scalar_tensor_tensor(aff2[:], th_t[c][:], 1.0,
                                           sim_sb[c][:], OP.add, OP.mult)
            ncoB = work.tile([P, C], F32, tag="ncoB")
            nc.vector.scalar_tensor_tensor(ncoB[:], co_t[c][:], SMOOTH,
                                           balu_t[c][:], OP.add, OP.mult)
            m_ = work.tile([P, C], F32, tag="m")
            nc.vector.tensor_tensor(m_[:], ncoB[:], aff2[:], OP.mult)
            pre = work.tile([P, C], F32, tag="pre", bufs=4)
            nc.vector.tensor_tensor(pre[:], m_[:], conf_t[c][:], OP.mult)
            E_t[c] = pre   # overwritten in-place by exp below

        def exp_chunk(c):
            # E = exp(pre) with row-sum accumulation (diag of pre is 0)
            Ec = work.tile([P, C], F32, tag="E", bufs=4)
            rs = tiny.tile([P, 1], F32, tag=f"rs{c}")
            nc.scalar.activation(Ec[:], E_t[c][:], AF.Exp, accum_out=rs[:])
            E_t[c] = Ec
            rs_t[c] = rs

        def bt_chunk(c):
            # diagP = diag(0.9/rowsum); Bt slices = E_c.T @ diagP (-0.9I)
            r09 = tiny.tile([P, 1], F32, tag=f"r09{c}")
            nc.vector.reciprocal(r09[:], rs_t[c][:])
            diagP = work.tile([P, P], F32, tag="diagP")
            nc.vector.tensor_scalar(diagP[:], id_t[:], r09[:], 0.9,
                                    OP.mult, OP.mult)
            for k in range(CT):
                tr_ps = psT.tile([P, P], F32, tag="tr")
                nc.tensor.matmul(tr_ps[:], E_t[c][:, bass.ts(k, P)],
                                 diagP[:], start=True, stop=True)
                dst = BtAll[:, bass.ts(k * CT + c, P)]
                if k == c:
                    nc.vector.tensor_tensor(dst, tr_ps[:], eye09[:],
                                            OP.subtract)
                elif k % 2 == 0:
                    nc.scalar.copy(dst, tr_ps[:])
                else:
                    nc.vector.tensor_copy(dst, tr_ps[:])

        def stage_b_m(m):
            # all 8 batches for output row-block m
            for pair in range(BPC // 2):
                d_ps = []
                for b2 in range(2):
                    dp = psB.tile([P, D], F32, tag="d", name=f"d{m}_{pair}_{b2}")
                    d_ps.append(dp)
                for k in range(CT):
                    w = BtAll[:, bass.ts(k * CT + m, P)]
                    for b2 in range(2):
                        for n in range(NT):
                            nc.tensor.matmul(
                                d_ps[b2][:, bass.ts(n, 512)], w,
                                xt_all[2 * pair + b2][k][:, bass.ts(n, 512)],
                                start=(k == 0), stop=(k == CT - 1))
                for b2 in range(2):
                    b = 2 * pair + b2
                    xm = xt_all[b][m]
                    d_bf = dsp.tile([P, D], BF16, tag="dsb")
                    nc.scalar.copy(d_bf[:], d_ps[b2][:])
                    ss = tbp.tile([P, 1], F32, tag="ss")
                    g1 = gbp.tile([P, D], BF16, tag="g")
                    if m == 0:
                        nc.scalar.activation(g1[:], xm[:], AF.Square,
                                             scale=SQ_SCALE, accum_out=ss[:])
                    else:
                        nc.vector.scalar_tensor_tensor(
                            g1[:], xm[:], INV32, xm[:], OP.mult, OP.mult,
                            accum_out=ss[:])
                    gs = tbp.tile([P, 1], F32, tag="gs")
                    g2 = gbp.tile([P, D], BF16, tag="g")
                    if m >= 2:
                        nc.vector.tensor_tensor(g2[:], xm[:], d_bf[:],
                                                OP.mult)
                        gsr = tbp.tile([P, 1], F32, tag="gsr")
                        nc.vector.tensor_reduce(gsr[:], g2[:], axis=AX.X,
                                                op=OP.add)
                        nc.vector.tensor_scalar(gs[:], gsr[:], INV32, None,
                                                OP.mult)
                    else:
                        nc.vector.scalar_tensor_tensor(
                            g2[:], xm[:], INV32, d_bf[:], OP.mult, OP.mult,
                            accum_out=gs[:])
                    gate = tbp.tile([P, 1], F32, tag="gate")
                    nc.scalar.activation(gate[:], gs[:], AF.Sigmoid,
                                         bias=ss[:])
                    o_t = obp.tile([P, D], BF16, tag="o")
                    nc.vector.scalar_tensor_tensor(
                        o_t[:], d_bf[:], gate[:], xm[:], OP.mult, OP.add)
                    nc.sync.dma_start(y_d[b, bass.ts(m, P), :], o_t[:])

        # ---------------- emission order ----------------
        # ACT set sequence: [sig: th0 conf0] [exp: E0] [sig: th1-3 conf1-3,
        # m0 squares/gates] [exp: E1-3] [sig: m1-3 gates]
        sim_chunk(0)
        th_conf_chunk(0)
        pre_chunk(0)
        exp_chunk(0)
        bt_chunk(0)
        for c in range(1, CT):
            sim_chunk(c)
            th_conf_chunk(c)
            pre_chunk(c)
        stage_b_m(0)
        for c in range(1, CT):
            exp_chunk(c)
            bt_chunk(c)
        for m in range(1, CT):
            stage_b_m(m)


LAST_RESULTS = None


def kernel(x, co_occurrence, class_counts, context_embeddings, _trace=False):
    global LAST_RESULTS
    if "nc" not in _CACHE:
        _CACHE["nc"] = _build_module()
    nc = _CACHE["nc"]

    import ml_dtypes
    x = np.ascontiguousarray(
        np.asarray(x, dtype=np.float32).astype(ml_dtypes.bfloat16))
    co = np.ascontiguousarray(np.asarray(co_occurrence, dtype=np.float32))
    cnt = np.asarray(class_counts, dtype=np.float64)
    emb = np.asarray(context_embeddings, dtype=np.float64)

    # host-side setup-scale precompute (counts/embeddings are tiny)
    s = SMOOTH
    avg = cnt.mean()
    minc = np.minimum(cnt[:, None], cnt[None, :])
    maxc = np.maximum(cnt[:, None], cnt[None, :])
    bal = np.where((minc > s) & (maxc > s),
                   np.log1p(maxc / avg) * (minc / maxc), s)
    u = (cnt + s) ** -0.5
    balu = 2.5 * bal * u[:, None] * u[None, :]
    np.fill_diagonal(balu, 0.0)
    balu = np.ascontiguousarray(balu.astype(np.float32))
    nemb = emb / np.linalg.norm(emb, axis=1, keepdims=True)
    nemb = np.ascontiguousarray(nemb.astype(np.float32))
    ident = np.eye(P, dtype=np.float32)

    in_maps = []
    for c in range(NCORES):
        in_maps.append({
            "x": x[c * BPC:(c + 1) * BPC],
            "co": co,
            "balu": balu,
            "nemb": nemb,
            "ident": ident,
        })
    res = run_bass_kernel_spmd(nc, in_maps, list(range(NCORES)), trace=_trace)
    LAST_RESULTS = res
    y = np.concatenate([r["y"] for r in res.results], axis=0)
    return np.ascontiguousarray(y.astype(np.float32))


# revision 10
# speedup vs baseline: 1.1993x; 1.1993x over previous
"""Trainium2 Bass kernel for nn_CoOccurrenceGraph.

Computation (full problem: B=64, C=512, D=1024):
    ew  = edge_weights(co_occurrence, class_counts, context_embeddings)  # [C,C]
    x_t = ew @ x[b]                          # per batch
    gate = sigmoid(sum(x*x_t, -1)/sqrt(D))   # [B,C,1]
    out  = x*(1-gate) + x_t*gate

Strategy: data-parallel over batch across 8 NeuronCores (8 batches/core).
Each core builds the full [C,C] edge-weight matrix on-device (replicated),
then runs the per-batch matmuls + gating.

V2 design (vs the 193 us baseline):
  * Host precomputes the tiny count-derived factor
    balu = 2.5 * balance * u_i * u_j * offdiag  ([C,C], ~1MB, setup-scale)
    and the normalized embeddings, killing most of the edge-weight build's
    on-device elementwise work and ACT-table thrash.
  * Per-chunk (128-row) edge-weight pipeline: the first chunk's weights
    reach the PE ~10us in; stage-B matmuls then overlap the remaining
    edge-weight build.
  * Softmax row-scaling rides the PE "transpose" as a matmul against
    diag(0.9/rowsum) - no separate scale pass.
  * sigmoid(z) = (tanh(... )+1)/2 keeps the whole affinity+confidence
    group in the sigmoid/tanh ACT table set; exp/softmax is the only
    other set -> 5 table loads total (vs 8), none on the critical path.
  * d = (ew - I)@x is copied PSUM->SBUF as bf16 so both DVE passes
    (q-accum and the gated combine) run in 2x packed mode.
  * Output is written bf16 (host casts back to f32): halves output DMA.
  * ss = sum(x^2) runs on ACT (Square, set-filler) for m==0 tiles and on
    DVE for the rest to balance the two engines.
"""

import numpy as np

import concourse.bass as bass
import concourse.bacc as bacc
import concourse.mybir as mybir
import concourse.tile as tile
from concourse.bass_utils import run_bass_kernel_spmd

F32 = mybir.dt.float32
BF16 = mybir.dt.bfloat16
AX = mybir.AxisListType
OP = mybir.AluOpType
AF = mybir.ActivationFunctionType

B, C, D = 64, 512, 1024
P = 128
NCORES = 8
BPC = B // NCORES          # batches per core
CT = C // P                # 4 chunks of 128 rows
NT = D // 512              # 2 matmul n-groups
SMOOTH = 0.01
INV32 = 1.0 / float(np.sqrt(D))      # 1/32
SQ_SCALE = float(D) ** -0.25         # Square(s*x) accumulates x^2/sqrt(D)

_CACHE = {}


def _build_module():
    nc = bacc.Bacc("TRN2", target_bir_lowering=False, debug=False,
                   num_devices=NCORES)
    x_d = nc.dram_tensor("x", [BPC, C, D], BF16, kind="ExternalInput").ap()
    co_d = nc.dram_tensor("co", [C, C], F32, kind="ExternalInput").ap()
    balu_d = nc.dram_tensor("balu", [C, C], F32, kind="ExternalInput").ap()
    nemb_d = nc.dram_tensor("nemb", [C, 4], F32, kind="ExternalInput").ap()
    id_d = nc.dram_tensor("ident", [P, P], F32, kind="ExternalInput").ap()
    y_d = nc.dram_tensor("y", [BPC, C, D], BF16, kind="ExternalOutput").ap()

    with tile.TileContext(nc) as tc:
        _body(nc, tc, x_d, co_d, balu_d, nemb_d, id_d, y_d)
    if not nc.is_finalized():
        nc.finalize()
    return nc


def _body(nc, tc, x_d, co_d, balu_d, nemb_d, id_d, y_d):
    from contextlib import ExitStack
    with ExitStack() as ctx:
        persist = ctx.enter_context(tc.tile_pool(name="persist", bufs=1))
        work = ctx.enter_context(tc.tile_pool(name="work", bufs=2))
        tiny = ctx.enter_context(tc.tile_pool(name="tiny", bufs=6))
        xbp = ctx.enter_context(tc.tile_pool(name="xb", bufs=32))
        dsp = ctx.enter_context(tc.tile_pool(name="ds", bufs=6))
        gbp = ctx.enter_context(tc.tile_pool(name="gb", bufs=3))
        obp = ctx.enter_context(tc.tile_pool(name="ob", bufs=4))
        tbp = ctx.enter_context(tc.tile_pool(name="tb", bufs=10))
        psS = ctx.enter_context(
            tc.tile_pool(name="psS", bufs=2, space="PSUM"))   # sim [P,C]
        psT = ctx.enter_context(
            tc.tile_pool(name="psT", bufs=2, space="PSUM"))   # tr [P,P]
        psB = ctx.enter_context(
            tc.tile_pool(name="psB", bufs=2, space="PSUM"))   # d [P,D]

        # ---------------- input DMAs (order = priority) ----------------
        id_t = persist.tile([P, P], F32, tag="ident")
        nc.sync.dma_start(id_t[:], id_d[:])
        e_t = []
        for c in range(CT):
            et = tiny.tile([P, 4], F32, tag=f"emb{c}")
            nc.sync.dma_start(et[:], nemb_d[bass.ts(c, P), :])
            e_t.append(et)
        co_t = []
        balu_t = []
        for c in range(CT):
            ct_ = persist.tile([P, C], F32, tag=f"co{c}")
            nc.sync.dma_start(ct_[:], co_d[bass.ts(c, P), :])
            co_t.append(ct_)
        for c in range(CT):
            bt_ = persist.tile([P, C], F32, tag=f"balu{c}")
            nc.sync.dma_start(bt_[:], balu_d[bass.ts(c, P), :])
            balu_t.append(bt_)
        xt_all = []
        for b in range(BPC):
            xt = []
            for k in range(CT):
                xk = xbp.tile([P, D], BF16, tag="x")
                nc.sync.dma_start(xk[:], x_d[b, bass.ts(k, P), :])
                xt.append(xk)
            xt_all.append(xt)

        # eye09 = 0.9*I (for the diagonal-block fix of A = 0.9*(sm - I))
        eye09 = persist.tile([P, P], F32, tag="eye09")
        nc.vector.tensor_scalar(eye09[:], id_t[:], 0.9, None, OP.mult)
        bm25 = persist.tile([P, 1], F32, tag="bm25")
        nc.vector.memset(bm25[:], -2.5)

        # nembT [4, C]: PE transposes of the host-normalized embeddings
        nembT = persist.tile([4, C], F32, tag="nembT")
        for c in range(CT):
            neT_ps = psT.tile([4, P], F32, tag="tr")
            nc.tensor.transpose(neT_ps[:], e_t[c][:], id_t[:])
            nc.scalar.copy(nembT[:, bass.ts(c, P)], neT_ps[:])

        # per-chunk stage-E state
        sim_sb = [None] * CT
        th_t = [None] * CT
        conf_t = [None] * CT
        E_t = [None] * CT
        rs_t = [None] * CT
        BtAll = persist.tile([P, CT * C], BF16, tag="BtAll")

        def sim_chunk(c):
            # sim_c stays in PSUM; read by ACT (th) and DVE (aff)
            s_ps = psS.tile([P, C], F32, tag="sim")
            nc.tensor.matmul(s_ps[:], nembT[:, bass.ts(c, P)], nembT[:],
                             start=True, stop=True)
            sim_sb[c] = s_ps

        def th_conf_chunk(c):
            # th = tanh(5*sim - 2.5); sigmoid(10(sim-.5)) = (th+1)/2
            th = work.tile([P, C], F32, tag="th", bufs=4)
            nc.scalar.activation(th[:], sim_sb[c][:], AF.Tanh,
                                 bias=bm25[:], scale=5.0)
            th_t[c] = th
            cf = work.tile([P, C], F32, tag="conf", bufs=4)
            nc.scalar.activation(cf[:], co_t[c][:], AF.Tanh, scale=0.1)
            conf_t[c] = cf

        def pre_chunk(c):
            # aff2 = (th+1)*sim ; ncoB = (co+s)*balu ; pre = ncoB*aff2*conf
            aff2 = work.tile([P, C], F32, tag="aff2")
            nc.vector.scalar_tensor_tensor(aff2[:], th_t[c][:], 1.0,
                                           sim_sb[c][:], OP.add, OP.mult)
            ncoB = work.tile([P, C], F32, tag="ncoB")
            nc.vector.scalar_tensor_tensor(ncoB[:], co_t[c][:], SMOOTH,
                                           balu_t[c][:], OP.add, OP.mult)
            m_ = work.tile([P, C], F32, tag="m")
            nc.vector.tensor_tensor(m_[:], ncoB[:], aff2[:], OP.mult)
            pre = work.tile([P, C], F32, tag="pre", bufs=4)
            nc.vector.tensor_tensor(pre[:], m_[:], conf_t[c][:], OP.mult)
            E_t[c] = pre   # overwritten in-place by exp below

        def exp_chunk(c):
            # E = exp(pre) with row-sum accumulation (diag of pre is 0)
            Ec = work.tile([P, C], F32, tag="E", bufs=4)
            rs = tiny.tile([P, 1], F32, tag=f"rs{c}")
            nc.scalar.activation(Ec[:], E_t[c][:], AF.Exp, accum_out=rs[:])
            E_t[c] = Ec
            rs_t[c] = rs

        def bt_chunk(c):
            # diagP = diag(0.9/rowsum); Bt slices = E_c.T @ diagP (-0.9I)
            r09 = tiny.tile([P, 1], F32, tag=f"r09{c}")
            nc.vector.reciprocal(r09[:], rs_t[c][:])
            diagP = work.tile([P, P], F32, tag="diagP")
            nc.vector.tensor_scalar(diagP[:], id_t[:], r09[:], 0.9,
                                    OP.mult, OP.mult)
            for k in range(CT):
                tr_ps = psT.tile([P, P], F32, tag="tr")
                nc.tensor.matmul(tr_ps[:], E_t[c][:, bass.ts(k, P)],
                                 diagP[:], start=True, stop=True)
                dst = BtAll[:, bass.ts(k * CT + c, P)]
                if k == c:
                    nc.vector.tensor_tensor(dst, tr_ps[:], eye09[:],
                                            OP.subtract)
                elif k % 2 == 0:
                    nc.scalar.copy(dst, tr_ps[:])
                else:
                    nc.vector.tensor_copy(dst, tr_ps[:])

        def stage_b_m(m):
            # all 8 batches for output row-block m
            for pair in range(BPC // 2):
                d_ps = []
                for b2 in range(2):
                    dp = psB.tile([P, D], F32, tag="d", name=f"d{m}_{pair}_{b2}")
                    d_ps.append(dp)
                for k in range(CT):
                    w = BtAll[:, bass.ts(k * CT + m, P)]
                    for b2 in range(2):
                        for n in range(NT):
                            nc.tensor.matmul(
                                d_ps[b2][:, bass.ts(n, 512)], w,
                                xt_all[2 * pair + b2][k][:, bass.ts(n, 512)],
                                start=(k == 0), stop=(k == CT - 1))
                for b2 in range(2):
                    b = 2 * pair + b2
                    xm = xt_all[b][m]
                    d_bf = dsp.tile([P, D], BF16, tag="dsb")
                    nc.scalar.copy(d_bf[:], d_ps[b2][:])
                    ss = tbp.tile([P, 1], F32, tag="ss")
                    g1 = gbp.tile([P, D], BF16, tag="g")
                    if m < 3:
                        nc.scalar.activation(g1[:], xm[:], AF.Square,
                                             scale=SQ_SCALE, accum_out=ss[:])
                    else:
                        nc.vector.scalar_tensor_tensor(
                            g1[:], xm[:], INV32, xm[:], OP.mult, OP.mult,
                            accum_out=ss[:])
                    gs = tbp.tile([P, 1], F32, tag="gs")
                    g2 = gbp.tile([P, D], BF16, tag="g")
                    nc.vector.scalar_tensor_tensor(
                        g2[:], xm[:], INV32, d_bf[:], OP.mult, OP.mult,
                        accum_out=gs[:])
                    gate = tbp.tile([P, 1], F32, tag="gate")
                    nc.scalar.activation(gate[:], gs[:], AF.Sigmoid,
                                         bias=ss[:])
                    o_t = obp.tile([P, D], BF16, tag="o")
                    nc.vector.scalar_tensor_tensor(
                        o_t[:], d_bf[:], gate[:], xm[:], OP.mult, OP.add)
                    nc.sync.dma_start(y_d[b, bass.ts(m, P), :], o_t[:])

        # ---------------- emission order ----------------
        # ACT set sequence: [sig: th0 conf0] [exp: E0] [sig: th1-3 conf1-3,
        # m0 squares/gates] [exp: E1-3] [sig: m1-3 gates]
        sim_chunk(0)
        th_conf_chunk(0)
        pre_chunk(0)
        exp_chunk(0)
        bt_chunk(0)
        for c in range(1, CT):
            sim_chunk(c)
            th_conf_chunk(c)
            pre_chunk(c)
        stage_b_m(0)
        for c in range(1, CT):
            exp_chunk(c)
            bt_chunk(c)
        for m in range(1, CT):
            stage_b_m(m)


LAST_RESULTS = None


def kernel(x, co_occurrence, class_counts, context_embeddings, _trace=False):
    global LAST_RESULTS
    if "nc" not in _CACHE:
        _CACHE["nc"] = _build_module()
    nc = _CACHE["nc"]

    import ml_dtypes
    x = np.ascontiguousarray(
        np.asarray(x, dtype=np.float32).astype(ml_dtypes.bfloat16))
    co = np.ascontiguousarray(np.asarray(co_occurrence, dtype=np.float32))
    cnt = np.asarray(class_counts, dtype=np.float64)
    emb = np.asarray(context_embeddings, dtype=np.float64)

    # host-side setup-scale precompute (counts/embeddings are tiny)
    s = SMOOTH
    avg = cnt.mean()
    minc = np.minimum(cnt[:, None], cnt[None, :])
    maxc = np.maximum(cnt[:, None], cnt[None, :])
    bal = np.where((minc > s) & (maxc > s),
                   np.log1p(maxc / avg) * (minc / maxc), s)
    u = (cnt + s) ** -0.5
    balu = 2.5 * bal * u[:, None] * u[None, :]
    np.fill_diagonal(balu, 0.0)
    balu = np.ascontiguousarray(balu.astype(np.float32))
    nemb = emb / np.linalg.norm(emb, axis=1, keepdims=True)
    nemb = np.ascontiguousarray(nemb.astype(np.float32))
    ident = np.eye(P, dtype=np.float32)

    in_maps = []
    for c in range(NCORES):
        in_maps.append({
            "x": x[c * BPC:(c + 1) * BPC],
            "co": co,
            "balu": balu,
            "nemb": nemb,
            "ident": ident,
        })
    res = run_bass_kernel_spmd(nc, in_maps, list(range(NCORES)), trace=_trace)
    LAST_RESULTS = res
    y = np.concatenate([r["y"] for r in res.results], axis=0)
    return np.ascontiguousarray(y.astype(np.float32))
